# revision 1
# baseline (speedup 1.0000x reference)
"""Bidirectional LSTM layer on 8 Trainium2 NeuronCores.

Sharding: each of the 8 cores owns ONE 128-unit eighth of the hidden dim for
BOTH directions (fwd and bwd).  The two directions' recurrences are
independent, so their sub-steps are interleaved: while direction f's serial
tail (gates -> c -> h -> transpose -> AllGather -> reload) is in flight, the
TensorEngine runs direction b's matmuls, and vice versa.  This keeps the PE
continuously busy (warm clock) and hides most of the per-step latency chain.

Per core, per sub-step (direction d, time t):
    z[64, 512] = x_t @ Wk_cols (+ b_cols) + h_{t-1} @ Wr_cols      (PSUM, 1 bank)
with column order [g | i | f | o] (each 128 wide).  h eighth is transposed
on the PE and AllGathered across all 8 cores (one group; rank r contributes
H rows [128r, 128r+128)), yielding the full hT [1024, 64] for the next step.
x-part matmuls are emitted 2 steps ahead (zps bufs=3) as PE filler work.
Matmuls run as float32r (full PE rate, ~1e-4 rel err).
"""

import sys

sys.path.insert(0, "/opt/trn_rl_repo")

import numpy as np

import concourse.bass as bass
import concourse.tile as tile
from concourse import bacc, mybir
from concourse.bass_utils import run_bass_kernel_spmd
from concourse.masks import make_identity

F32 = mybir.dt.float32
F32R = mybir.dt.float32r
ACT = mybir.ActivationFunctionType

B = 64       # batch
D = 512      # input dim
H = 1024     # hidden dim
HE = H // 8  # per-core hidden eighth
NCOL = 4 * HE  # 512 z-columns per direction per core
NC = 8

# K-chunk layout of the combined weight matrix W[d] [13*128, 512]:
#   chunks 0..3   : Wk rows (D = 512)
#   chunk  4      : row 0 = bias, rest zero
#   chunks 5..12  : Wr rows (H = 1024)
KC_X = 4
KC_H = 8
W_ROWS = (KC_X + 1 + KC_H) * 128


def build(T: int, mode: str = "full", has_bias: bool = False):
    nc = bacc.Bacc("TRN2", target_bir_lowering=False, debug=False, num_devices=NC)

    xp_d = nc.dram_tensor("xp", [2, T, D, B], F32R, kind="ExternalInput")
    w_d = nc.dram_tensor("W", [2, W_ROWS, NCOL], F32R, kind="ExternalInput")
    ones_d = nc.dram_tensor("ones", [1, B], F32R, kind="ExternalInput")
    out_d = nc.dram_tensor("hs", [2, T, B, HE], F32, kind="ExternalOutput")

    with tile.TileContext(nc) as tc:
        with (
            tc.tile_pool(name="singles", bufs=1) as singles,
            tc.tile_pool(name="xpool", bufs=4) as xpool,
            tc.tile_pool(name="hpool", bufs=3) as hpool,
            tc.tile_pool(name="apool", bufs=3) as apool,
            tc.tile_pool(name="cpool", bufs=2) as cpool,
            tc.tile_pool(name="spool", bufs=4) as spool,
            tc.tile_pool(name="zps", bufs=3, space="PSUM") as zps,
            tc.tile_pool(name="tps", bufs=2, space="PSUM") as tps,
            tc.tile_pool(name="dram", bufs=3, space="DRAM") as dram,
        ):
            # --- persistent tiles ------------------------------------------
            w_sb = singles.tile([128, 2, KC_X + 1 + KC_H, NCOL], F32R)
            nc.sync.dma_start(w_sb, w_d.ap().rearrange("d (n p) m -> p d n m", p=128))

            ident = singles.tile([128, 128], F32)
            make_identity(nc, ident)

            ones = singles.tile([1, B], F32R)
            nc.sync.dma_start(ones, ones_d.ap())

            st = []  # per-direction state
            for d in range(2):
                c0 = cpool.tile([B, HE], F32, tag=f"c{d}")
                nc.vector.memset(c0, 0.0)
                st.append({"c": c0, "hT": None, "z": {}, "h": {}})

            XB = 4  # steps per batched x load / h store
            HIPRI = 40  # pull exchange-path instructions ahead in the scheduler

            def emit_xpart(d, t):
                """x_t @ Wk (+ bias) into a fresh z psum tile; no h deps."""
                if t >= T:
                    return
                if t % XB == 0:  # batched load of XB steps of x (transposed)
                    nsteps = min(XB, T - t)
                    xT = xpool.tile([128, XB, KC_X, B], F32R, tag=f"xT{d}")
                    nc.scalar.dma_start(
                        xT[:, 0:nsteps],
                        xp_d.ap()[d, t:t + nsteps].rearrange(
                            "t (n p) m -> p t n m", p=128))
                    st[d]["xT"] = xT
                xT = st[d]["xT"]
                z = zps.tile([B, NCOL], F32, tag=f"z{d}")
                st[d]["z"][t] = z
                last_x = (t == 0) and not has_bias
                for k in range(KC_X):
                    nc.tensor.matmul(z, xT[:, t % XB, k, :], w_sb[:, d, k, :],
                                     start=(k == 0),
                                     stop=(last_x and k == KC_X - 1))
                if has_bias:
                    nc.tensor.matmul(z, ones[0:1, :], w_sb[0:1, d, KC_X, :],
                                     start=False, stop=(t == 0))

            def emit_hmm(d, t):
                z = st[d]["z"][t]
                if t > 0:
                    hT = st[d]["hT"]
                    for k in range(KC_H):
                        nc.tensor.matmul(z, hT[:, k, :],
                                         w_sb[:, d, KC_X + 1 + k, :],
                                         start=False, stop=(k == KC_H - 1))

            def emit_gates(d, t):
                """a = [tanh(g), sig(i), sig(f,o)] + ig = i*g."""
                z = st[d]["z"].pop(t)
                a = apool.tile([B, NCOL], F32, tag=f"a{d}", name=f"a{d}")
                nc.scalar.activation(a[:, 0:HE], z[:, 0:HE], ACT.Tanh)
                nc.scalar.activation(a[:, HE:2 * HE], z[:, HE:2 * HE], ACT.Sigmoid)
                nc.scalar.activation(a[:, 2 * HE:4 * HE], z[:, 2 * HE:4 * HE],
                                     ACT.Sigmoid)
                ig = spool.tile([B, HE], F32, tag=f"ig{d}", name=f"ig{d}")
                nc.vector.tensor_mul(ig, a[:, HE:2 * HE], a[:, 0:HE])
                st[d]["a"], st[d]["ig"] = a, ig

            def emit_cup(d, t):
                """c_new = f*c + ig."""
                a = st[d]["a"]
                fc = spool.tile([B, HE], F32, tag=f"fc{d}", name=f"fc{d}")
                nc.vector.tensor_mul(fc, a[:, 2 * HE:3 * HE], st[d]["c"])
                c_new = cpool.tile([B, HE], F32, tag=f"c{d}", name=f"c{d}")
                nc.vector.tensor_add(c_new, fc, st[d]["ig"])
                st[d]["c"] = c_new

            def emit_tanhc(d, t):
                th = spool.tile([B, HE], F32, tag=f"th{d}", name=f"th{d}")
                nc.scalar.activation(th, st[d]["c"], ACT.Tanh)
                st[d]["th"] = th

            def emit_hmul(d, t):
                if t % XB == 0:  # h ring buffer, stored XB steps at a time
                    st[d]["hr"] = spool.tile([B, XB, HE], F32, tag=f"hr{d}",
                                             name=f"hr{d}")
                hr = st[d]["hr"]
                h = hr[:, t % XB, :]
                nc.vector.tensor_mul(h, st[d]["a"][:, 3 * HE:4 * HE], st[d]["th"])
                st[d]["h"][t] = h
                if t % XB == XB - 1 or t == T - 1:
                    t0 = t - t % XB
                    nc.gpsimd.dma_start(
                        out_d.ap()[d, t0:t + 1].rearrange("t b e -> b t e"),
                        hr[:, 0:t - t0 + 1, :])

            def emit_transpose(d, t):
                if t == T - 1:
                    return
                h = st[d]["h"].pop(t)
                with tc.high_priority(offset=HIPRI):
                    tp = tps.tile([128, B], F32, tag="tp")
                    nc.tensor.transpose(tp, h, ident[0:B, 0:B])
                    hTs = spool.tile([128, B], F32R, tag=f"hTs{d}", name=f"hTs{d}")
                    nc.vector.tensor_copy(hTs, tp)
                    ag_in = dram.tile([128, B], F32R, tag=f"ag_in{d}",
                                      name=f"ag_in{d}")
                    nc.sync.dma_start(ag_in, hTs)
                st[d]["ag_in"] = ag_in

            def emit_ag(d, t):
                if t == T - 1:
                    return
                ag_in = st[d]["ag_in"]
                ag_out = dram.tile([KC_H * 128, B], F32R, tag=f"ag_out{d}",
                                   name=f"ag_out{d}")
                if mode == "noag":
                    # timing stand-in for the AllGather (2 local DMAs;
                    # rows 256+ stay stale — noag is timing-only)
                    nc.sync.dma_start(ag_out[0:128, :], ag_in)
                    nc.sync.dma_start(ag_out[128:256, :], ag_in)
                else:
                    nc.gpsimd.collective_compute(
                        "AllGather", mybir.AluOpType.bypass,
                        replica_groups=[list(range(NC))],
                        ins=[ag_in.opt()], outs=[ag_out.opt()],
                    )
                st[d]["ag_out"] = ag_out

            def emit_hload(d, t):
                if t == T - 1:
                    return
                hT = hpool.tile([128, KC_H, B], F32R, tag=f"hT{d}", name=f"hT{d}")
                with tc.high_priority(offset=HIPRI):
                    nc.sync.dma_start(
                        hT, st[d]["ag_out"].rearrange("(n p) m -> p n m", p=128))
                st[d]["hT"] = hT

            emit_xpart(0, 0)
            emit_xpart(1, 0)
            emit_xpart(0, 1)
            emit_xpart(1, 1)
            for t in range(T):
                emit_hmm(0, t)
                emit_xpart(0, t + 2)
                emit_gates(0, t)
                emit_cup(0, t)
                emit_tanhc(0, t)
                emit_hmm(1, t)
                emit_hmul(0, t)
                emit_transpose(0, t)
                emit_ag(0, t)
                emit_hload(0, t)
                emit_gates(1, t)
                emit_cup(1, t)
                emit_tanhc(1, t)
                emit_hmul(1, t)
                emit_xpart(1, t + 2)
                emit_transpose(1, t)
                emit_ag(1, t)
                emit_hload(1, t)

    nc.compile()
    return nc


def make_in_maps(x, Wk_f, Wr_f, b_f, Wk_b, Wr_b, b_b):
    """Host-side prep: transpose x, slice/reorder weight columns per core."""
    T = x.shape[1]
    xp = np.empty((2, T, D, B), np.float32)
    xp[0] = np.transpose(x, (1, 2, 0))  # [T, D, B]
    xp[1] = xp[0][::-1]
    in_maps = []
    for c in range(NC):
        W = np.zeros((2, W_ROWS, NCOL), np.float32)
        sl = c * HE + np.arange(HE)
        # column order [g, i, f, o]  (keras gate order in W is i,f,g,o)
        cols = np.concatenate([2 * H + sl, 0 * H + sl, 1 * H + sl, 3 * H + sl])
        for d, (Wk, Wr, b) in enumerate([(Wk_f, Wr_f, b_f), (Wk_b, Wr_b, b_b)]):
            W[d, 0:D] = np.asarray(Wk)[:, cols]
            W[d, D] = np.asarray(b)[cols]
            W[d, (KC_X + 1) * 128:] = np.asarray(Wr)[:, cols]
        in_maps.append({"xp": xp, "W": W, "ones": np.ones((1, B), np.float32)})
    return in_maps


def combine(results, T):
    """Gather per-core [2, T, B, HE] outputs into [B, T, H]."""
    out = np.zeros((B, T, H), np.float32)
    for c in range(NC):
        hs = results[c]["hs"]  # [2, T, B, HE]
        f = np.transpose(hs[0], (1, 0, 2))          # [B, T, HE]
        bwd = np.transpose(hs[1], (1, 0, 2))[:, ::-1]
        out[:, :, c * HE:(c + 1) * HE] = 0.5 * (f + bwd)
    return out


_NC_CACHE = {}


def run(x, Wk_f, Wr_f, b_f, Wk_b, Wr_b, b_b, trace=False, **spmd_kwargs):
    T = x.shape[1]
    has_bias = bool(np.any(b_f) or np.any(b_b))
    key = (T, has_bias)
    if key not in _NC_CACHE:
        _NC_CACHE[key] = build(T, has_bias=has_bias)
    nc = _NC_CACHE[key]
    in_maps = make_in_maps(x, Wk_f, Wr_f, b_f, Wk_b, Wr_b, b_b)
    res = run_bass_kernel_spmd(nc, in_maps, core_ids=list(range(NC)),
                               trace=trace, **spmd_kwargs)
    return combine(res.results, T), res


def kernel(x, Wk_f, Wr_f, b_f, Wk_b, Wr_b, b_b):
    out, _ = run(np.asarray(x), np.asarray(Wk_f), np.asarray(Wr_f), np.asarray(b_f),
                 np.asarray(Wk_b), np.asarray(Wr_b), np.asarray(b_b))
    return out



# revision 2
# speedup vs baseline: 1.3097x; 1.3097x over previous
"""Bidirectional LSTM layer on 8 Trainium2 NeuronCores.

Sharding: each of the 8 cores owns ONE 128-unit eighth of the hidden dim for
BOTH directions (fwd and bwd).  The two directions' recurrences are
independent, so their sub-steps are interleaved: while direction f's serial
tail (gates -> c -> h -> transpose -> AllGather -> reload) is in flight, the
TensorEngine runs direction b's matmuls, and vice versa.  This keeps the PE
continuously busy (warm clock) and hides most of the per-step latency chain.

Per core, per sub-step (direction d, time t):
    z[64, 512] = x_t @ Wk_cols (+ b_cols) + h_{t-1} @ Wr_cols      (PSUM, 1 bank)
with column order [g | i | f | o] (each 128 wide).  h eighth is transposed
on the PE and AllGathered across all 8 cores (one group; rank r contributes
H rows [128r, 128r+128)), yielding the full hT [1024, 64] for the next step.
x-part matmuls are emitted 2 steps ahead (zps bufs=3) as PE filler work.
Matmuls run as float32r (full PE rate); the h recurrence matmuls and the
per-step h exchange (transpose -> AllGather -> reload) run in bfloat16,
halving the collective and reload bytes (~1e-3 rel err, well inside the
2e-2 gate).
"""

import sys

sys.path.insert(0, "/opt/trn_rl_repo")

import numpy as np

import concourse.bass as bass
import concourse.tile as tile
from concourse import bacc, mybir
from concourse.bass_utils import run_bass_kernel_spmd
from concourse.masks import make_identity

F32 = mybir.dt.float32
F32R = mybir.dt.float32r
BF16 = mybir.dt.bfloat16
ACT = mybir.ActivationFunctionType

B = 64       # batch
D = 512      # input dim
H = 1024     # hidden dim
HE = H // 8  # per-core hidden eighth
NCOL = 4 * HE  # 512 z-columns per direction per core
NC = 8

# K-chunk layout of the combined weight matrix W[d] [13*128, 512]:
#   chunks 0..3   : Wk rows (D = 512)
#   chunk  4      : row 0 = bias, rest zero
#   chunks 5..12  : Wr rows (H = 1024)
KC_X = 4
KC_H = 8
W_ROWS = (KC_X + 1 + KC_H) * 128


def build(T: int, mode: str = "full", has_bias: bool = False):
    nc = bacc.Bacc("TRN2", target_bir_lowering=False, debug=False, num_devices=NC)

    xp_d = nc.dram_tensor("xp", [2, T, D, B], F32R, kind="ExternalInput")
    w_d = nc.dram_tensor("W", [2, W_ROWS, NCOL], F32R, kind="ExternalInput")
    wh_d = nc.dram_tensor("Wh", [2, KC_H * 128, NCOL], BF16, kind="ExternalInput")
    ones_d = nc.dram_tensor("ones", [1, B], F32R, kind="ExternalInput")
    out_d = nc.dram_tensor("hs", [2, T, B, HE], F32, kind="ExternalOutput")

    with tile.TileContext(nc) as tc:
        with (
            tc.tile_pool(name="singles", bufs=1) as singles,
            tc.tile_pool(name="xpool", bufs=4) as xpool,
            tc.tile_pool(name="hpool", bufs=3) as hpool,
            tc.tile_pool(name="apool", bufs=3) as apool,
            tc.tile_pool(name="cpool", bufs=2) as cpool,
            tc.tile_pool(name="spool", bufs=4) as spool,
            tc.tile_pool(name="zps", bufs=3, space="PSUM") as zps,
            tc.tile_pool(name="tps", bufs=2, space="PSUM") as tps,
            tc.tile_pool(name="dram", bufs=3, space="DRAM") as dram,
        ):
            # --- persistent tiles ------------------------------------------
            w_sb = singles.tile([128, 2, KC_X + 1 + KC_H, NCOL], F32R)
            nc.sync.dma_start(w_sb, w_d.ap().rearrange("d (n p) m -> p d n m", p=128))

            wh_sb = singles.tile([128, 2, KC_H, NCOL], BF16)
            nc.sync.dma_start(wh_sb,
                              wh_d.ap().rearrange("d (n p) m -> p d n m", p=128))

            ident = singles.tile([128, 128], F32)
            make_identity(nc, ident)

            ones = singles.tile([1, B], F32R)
            nc.sync.dma_start(ones, ones_d.ap())

            st = []  # per-direction state
            for d in range(2):
                c0 = cpool.tile([B, HE], F32, tag=f"c{d}")
                nc.vector.memset(c0, 0.0)
                st.append({"c": c0, "hT": None, "z": {}, "h": {}})

            XB = 4  # steps per batched x load / h store
            HIPRI = 40  # pull exchange-path instructions ahead in the scheduler

            def emit_xpart(d, t):
                """x_t @ Wk (+ bias) into a fresh z psum tile; no h deps."""
                if t >= T:
                    return
                if t % XB == 0:  # batched load of XB steps of x (transposed)
                    nsteps = min(XB, T - t)
                    xT = xpool.tile([128, XB, KC_X, B], F32R, tag=f"xT{d}")
                    nc.scalar.dma_start(
                        xT[:, 0:nsteps],
                        xp_d.ap()[d, t:t + nsteps].rearrange(
                            "t (n p) m -> p t n m", p=128))
                    st[d]["xT"] = xT
                xT = st[d]["xT"]
                z = zps.tile([B, NCOL], F32, tag=f"z{d}")
                st[d]["z"][t] = z
                last_x = (t == 0) and not has_bias
                for k in range(KC_X):
                    nc.tensor.matmul(z, xT[:, t % XB, k, :], w_sb[:, d, k, :],
                                     start=(k == 0),
                                     stop=(last_x and k == KC_X - 1))
                if has_bias:
                    nc.tensor.matmul(z, ones[0:1, :], w_sb[0:1, d, KC_X, :],
                                     start=False, stop=(t == 0))

            def emit_hmm(d, t):
                z = st[d]["z"][t]
                if t > 0:
                    hT = st[d]["hT"]
                    for k in range(KC_H):
                        nc.tensor.matmul(z, hT[:, k, :],
                                         wh_sb[:, d, k, :],
                                         start=False, stop=(k == KC_H - 1))

            def emit_gates(d, t):
                """a = [tanh(g), sig(i), sig(f,o)] + ig = i*g."""
                z = st[d]["z"].pop(t)
                a = apool.tile([B, NCOL], F32, tag=f"a{d}", name=f"a{d}")
                nc.scalar.activation(a[:, 0:HE], z[:, 0:HE], ACT.Tanh)
                nc.scalar.activation(a[:, HE:2 * HE], z[:, HE:2 * HE], ACT.Sigmoid)
                nc.scalar.activation(a[:, 2 * HE:4 * HE], z[:, 2 * HE:4 * HE],
                                     ACT.Sigmoid)
                ig = spool.tile([B, HE], F32, tag=f"ig{d}", name=f"ig{d}")
                nc.vector.tensor_mul(ig, a[:, HE:2 * HE], a[:, 0:HE])
                st[d]["a"], st[d]["ig"] = a, ig

            def emit_cup(d, t):
                """c_new = f*c + ig."""
                a = st[d]["a"]
                fc = spool.tile([B, HE], F32, tag=f"fc{d}", name=f"fc{d}")
                nc.vector.tensor_mul(fc, a[:, 2 * HE:3 * HE], st[d]["c"])
                c_new = cpool.tile([B, HE], F32, tag=f"c{d}", name=f"c{d}")
                nc.vector.tensor_add(c_new, fc, st[d]["ig"])
                st[d]["c"] = c_new

            def emit_tanhc(d, t):
                th = spool.tile([B, HE], F32, tag=f"th{d}", name=f"th{d}")
                nc.scalar.activation(th, st[d]["c"], ACT.Tanh)
                st[d]["th"] = th

            def emit_hmul(d, t):
                if t % XB == 0:  # h ring buffer, stored XB steps at a time
                    st[d]["hr"] = spool.tile([B, XB, HE], F32, tag=f"hr{d}",
                                             name=f"hr{d}")
                hr = st[d]["hr"]
                h = hr[:, t % XB, :]
                nc.vector.tensor_mul(h, st[d]["a"][:, 3 * HE:4 * HE], st[d]["th"])
                st[d]["h"][t] = h
                if t % XB == XB - 1 or t == T - 1:
                    t0 = t - t % XB
                    nc.gpsimd.dma_start(
                        out_d.ap()[d, t0:t + 1].rearrange("t b e -> b t e"),
                        hr[:, 0:t - t0 + 1, :])

            def emit_transpose(d, t):
                if t == T - 1:
                    return
                h = st[d]["h"].pop(t)
                with tc.high_priority(offset=HIPRI):
                    tp = tps.tile([128, B], F32, tag="tp")
                    nc.tensor.transpose(tp, h, ident[0:B, 0:B])
                    hTs = spool.tile([128, B], BF16, tag=f"hTs{d}", name=f"hTs{d}")
                    nc.vector.tensor_copy(hTs, tp)
                    ag_in = dram.tile([128, B], BF16, tag=f"ag_in{d}",
                                      name=f"ag_in{d}")
                    nc.sync.dma_start(ag_in, hTs)
                st[d]["ag_in"] = ag_in

            def emit_ag(d, t):
                if t == T - 1:
                    return
                ag_in = st[d]["ag_in"]
                ag_out = dram.tile([KC_H * 128, B], BF16, tag=f"ag_out{d}",
                                   name=f"ag_out{d}")
                if mode == "noag":
                    # timing stand-in for the AllGather (2 local DMAs;
                    # rows 256+ stay stale — noag is timing-only)
                    nc.sync.dma_start(ag_out[0:128, :], ag_in)
                    nc.sync.dma_start(ag_out[128:256, :], ag_in)
                else:
                    nc.gpsimd.collective_compute(
                        "AllGather", mybir.AluOpType.bypass,
                        replica_groups=[list(range(NC))],
                        ins=[ag_in.opt()], outs=[ag_out.opt()],
                    )
                st[d]["ag_out"] = ag_out

            def emit_hload(d, t):
                if t == T - 1:
                    return
                hT = hpool.tile([128, KC_H, B], BF16, tag=f"hT{d}", name=f"hT{d}")
                with tc.high_priority(offset=HIPRI):
                    nc.sync.dma_start(
                        hT, st[d]["ag_out"].rearrange("(n p) m -> p n m", p=128))
                st[d]["hT"] = hT

            emit_xpart(0, 0)
            emit_xpart(1, 0)
            emit_xpart(0, 1)
            emit_xpart(1, 1)
            for t in range(T):
                emit_hmm(0, t)
                emit_xpart(0, t + 2)
                emit_gates(0, t)
                emit_cup(0, t)
                emit_tanhc(0, t)
                emit_hmm(1, t)
                emit_hmul(0, t)
                emit_transpose(0, t)
                emit_ag(0, t)
                emit_hload(0, t)
                emit_gates(1, t)
                emit_cup(1, t)
                emit_tanhc(1, t)
                emit_hmul(1, t)
                emit_xpart(1, t + 2)
                emit_transpose(1, t)
                emit_ag(1, t)
                emit_hload(1, t)

    nc.compile()
    return nc


def make_in_maps(x, Wk_f, Wr_f, b_f, Wk_b, Wr_b, b_b):
    """Host-side prep: transpose x, slice/reorder weight columns per core."""
    T = x.shape[1]
    xp = np.empty((2, T, D, B), np.float32)
    xp[0] = np.transpose(x, (1, 2, 0))  # [T, D, B]
    xp[1] = xp[0][::-1]
    in_maps = []
    for c in range(NC):
        W = np.zeros((2, W_ROWS, NCOL), np.float32)
        sl = c * HE + np.arange(HE)
        # column order [g, i, f, o]  (keras gate order in W is i,f,g,o)
        cols = np.concatenate([2 * H + sl, 0 * H + sl, 1 * H + sl, 3 * H + sl])
        for d, (Wk, Wr, b) in enumerate([(Wk_f, Wr_f, b_f), (Wk_b, Wr_b, b_b)]):
            W[d, 0:D] = np.asarray(Wk)[:, cols]
            W[d, D] = np.asarray(b)[cols]
            W[d, (KC_X + 1) * 128:] = np.asarray(Wr)[:, cols]
        import ml_dtypes
        wh = W[:, (KC_X + 1) * 128:].astype(ml_dtypes.bfloat16)
        in_maps.append({"xp": xp, "W": W, "Wh": wh,
                        "ones": np.ones((1, B), np.float32)})
    return in_maps


def combine(results, T):
    """Gather per-core [2, T, B, HE] outputs into [B, T, H]."""
    out = np.zeros((B, T, H), np.float32)
    for c in range(NC):
        hs = results[c]["hs"]  # [2, T, B, HE]
        f = np.transpose(hs[0], (1, 0, 2))          # [B, T, HE]
        bwd = np.transpose(hs[1], (1, 0, 2))[:, ::-1]
        out[:, :, c * HE:(c + 1) * HE] = 0.5 * (f + bwd)
    return out


_NC_CACHE = {}


def run(x, Wk_f, Wr_f, b_f, Wk_b, Wr_b, b_b, trace=False, **spmd_kwargs):
    T = x.shape[1]
    has_bias = bool(np.any(b_f) or np.any(b_b))
    key = (T, has_bias)
    if key not in _NC_CACHE:
        _NC_CACHE[key] = build(T, has_bias=has_bias)
    nc = _NC_CACHE[key]
    in_maps = make_in_maps(x, Wk_f, Wr_f, b_f, Wk_b, Wr_b, b_b)
    res = run_bass_kernel_spmd(nc, in_maps, core_ids=list(range(NC)),
                               trace=trace, **spmd_kwargs)
    return combine(res.results, T), res


def kernel(x, Wk_f, Wr_f, b_f, Wk_b, Wr_b, b_b):
    out, _ = run(np.asarray(x), np.asarray(Wk_f), np.asarray(Wr_f), np.asarray(b_f),
                 np.asarray(Wk_b), np.asarray(Wr_b), np.asarray(b_b))
    return out



# revision 3
# speedup vs baseline: 1.6033x; 1.2242x over previous
"""Bidirectional LSTM layer on 8 Trainium2 NeuronCores.

Sharding: each of the 8 cores owns ONE 128-unit eighth of the hidden dim for
BOTH directions (fwd and bwd).  The two directions' recurrences are
independent, so their sub-steps are interleaved: while direction f's serial
tail (gates -> c -> h -> transpose -> AllGather -> reload) is in flight, the
TensorEngine runs direction b's matmuls, and vice versa.  This keeps the PE
continuously busy (warm clock) and hides most of the per-step latency chain.

Per core, per sub-step (direction d, time t):
    z[64, 512] = x_t @ Wk_cols (+ b_cols) + h_{t-1} @ Wr_cols      (PSUM, 1 bank)
with column order [g | i | f | o] (each 128 wide).  h eighth is transposed
on the PE and AllGathered across all 8 cores (one group; rank r contributes
H rows [128r, 128r+128)), yielding the full hT [1024, 64] for the next step.
x-part matmuls are emitted 2 steps ahead (zps bufs=3) as PE filler work.
Matmuls run as float32r (full PE rate); the h recurrence matmuls and the
per-step h exchange (transpose -> AllGather -> reload) run in bfloat16,
halving the collective and reload bytes (~1e-3 rel err, well inside the
2e-2 gate).
"""

import sys

sys.path.insert(0, "/opt/trn_rl_repo")

import numpy as np

import concourse.bass as bass
import concourse.tile as tile
from concourse import bacc, mybir
from concourse.bass_utils import run_bass_kernel_spmd
from concourse.masks import make_identity

F32 = mybir.dt.float32
F32R = mybir.dt.float32r
BF16 = mybir.dt.bfloat16
ACT = mybir.ActivationFunctionType

B = 64       # batch
D = 512      # input dim
H = 1024     # hidden dim
HE = H // 8  # per-core hidden eighth
NCOL = 4 * HE  # 512 z-columns per direction per core
NC = 8

# K-chunk layout of the combined weight matrix W[d] [13*128, 512]:
#   chunks 0..3   : Wk rows (D = 512)
#   chunk  4      : row 0 = bias, rest zero
#   chunks 5..12  : Wr rows (H = 1024)
KC_X = 4
KC_H = 8
W_ROWS = (KC_X + 1 + KC_H) * 128


def build(T: int, mode: str = "full", has_bias: bool = False):
    nc = bacc.Bacc("TRN2", target_bir_lowering=False, debug=False, num_devices=NC)

    xp_d = nc.dram_tensor("xp", [2, T, D, B], F32R, kind="ExternalInput")
    w_d = nc.dram_tensor("W", [2, W_ROWS, NCOL], F32R, kind="ExternalInput")
    wh_d = nc.dram_tensor("Wh", [2, KC_H * 128, NCOL], BF16, kind="ExternalInput")
    ones_d = nc.dram_tensor("ones", [1, B], F32R, kind="ExternalInput")
    out_d = nc.dram_tensor("hs", [2, T, B, HE], F32, kind="ExternalOutput")

    with tile.TileContext(nc) as tc:
        with (
            tc.tile_pool(name="singles", bufs=1) as singles,
            tc.tile_pool(name="xpool", bufs=4) as xpool,
            tc.tile_pool(name="hpool", bufs=3) as hpool,
            tc.tile_pool(name="apool", bufs=3) as apool,
            tc.tile_pool(name="cpool", bufs=2) as cpool,
            tc.tile_pool(name="spool", bufs=4) as spool,
            tc.tile_pool(name="zps", bufs=3, space="PSUM") as zps,
            tc.tile_pool(name="tps", bufs=2, space="PSUM") as tps,
            tc.tile_pool(name="dram", bufs=3, space="DRAM") as dram,
        ):
            # --- persistent tiles ------------------------------------------
            w_sb = singles.tile([128, 2, KC_X + 1 + KC_H, NCOL], F32R)
            nc.sync.dma_start(w_sb, w_d.ap().rearrange("d (n p) m -> p d n m", p=128))

            wh_sb = singles.tile([128, 2, KC_H, NCOL], BF16)
            nc.sync.dma_start(wh_sb,
                              wh_d.ap().rearrange("d (n p) m -> p d n m", p=128))

            ident = singles.tile([128, 128], F32)
            make_identity(nc, ident)

            ones = singles.tile([1, B], F32R)
            nc.sync.dma_start(ones, ones_d.ap())

            st = []  # per-direction state
            for d in range(2):
                c0 = cpool.tile([B, HE], F32, tag=f"c{d}")
                nc.vector.memset(c0, 0.0)
                st.append({"c": c0, "hT": None, "z": {}, "h": {}})

            XB = 4  # steps per batched x load / h store
            HIPRI = 100  # pull exchange-path instructions ahead in the scheduler

            def emit_xpart(d, t):
                """x_t @ Wk (+ bias) into a fresh z psum tile; no h deps."""
                if t >= T:
                    return
                if t % XB == 0:  # batched load of XB steps of x (transposed)
                    nsteps = min(XB, T - t)
                    xT = xpool.tile([128, XB, KC_X, B], F32R, tag=f"xT{d}")
                    nc.scalar.dma_start(
                        xT[:, 0:nsteps],
                        xp_d.ap()[d, t:t + nsteps].rearrange(
                            "t (n p) m -> p t n m", p=128))
                    st[d]["xT"] = xT
                xT = st[d]["xT"]
                z = zps.tile([B, NCOL], F32, tag=f"z{d}")
                st[d]["z"][t] = z
                last_x = (t == 0) and not has_bias
                for k in range(KC_X):
                    nc.tensor.matmul(z, xT[:, t % XB, k, :], w_sb[:, d, k, :],
                                     start=(k == 0),
                                     stop=(last_x and k == KC_X - 1))
                if has_bias:
                    nc.tensor.matmul(z, ones[0:1, :], w_sb[0:1, d, KC_X, :],
                                     start=False, stop=(t == 0))

            def emit_hmm(d, t):
                z = st[d]["z"][t]
                if t > 0:
                    hT = st[d]["hT"]
                    for k in range(KC_H):
                        nc.tensor.matmul(z, hT[:, k, :],
                                         wh_sb[:, d, k, :],
                                         start=False, stop=(k == KC_H - 1))

            def emit_gates(d, t):
                """a = [tanh(g), sig(i), sig(f,o)] + ig = i*g."""
                z = st[d]["z"].pop(t)
                a = apool.tile([B, NCOL], F32, tag=f"a{d}", name=f"a{d}")
                nc.scalar.activation(a[:, 0:HE], z[:, 0:HE], ACT.Tanh)
                nc.scalar.activation(a[:, HE:4 * HE], z[:, HE:4 * HE],
                                     ACT.Sigmoid)
                ig = spool.tile([B, HE], F32, tag=f"ig{d}", name=f"ig{d}")
                nc.vector.tensor_mul(ig, a[:, HE:2 * HE], a[:, 0:HE])
                st[d]["a"], st[d]["ig"] = a, ig

            def emit_cup(d, t):
                """c_new = f*c + ig."""
                a = st[d]["a"]
                fc = spool.tile([B, HE], F32, tag=f"fc{d}", name=f"fc{d}")
                nc.vector.tensor_mul(fc, a[:, 2 * HE:3 * HE], st[d]["c"])
                c_new = cpool.tile([B, HE], F32, tag=f"c{d}", name=f"c{d}")
                nc.vector.tensor_add(c_new, fc, st[d]["ig"])
                st[d]["c"] = c_new

            def emit_tanhc(d, t):
                th = spool.tile([B, HE], F32, tag=f"th{d}", name=f"th{d}")
                nc.scalar.activation(th, st[d]["c"], ACT.Tanh)
                st[d]["th"] = th

            def emit_hmul(d, t):
                if t % XB == 0:  # h ring buffer, stored XB steps at a time
                    st[d]["hr"] = spool.tile([B, XB, HE], F32, tag=f"hr{d}",
                                             name=f"hr{d}")
                hr = st[d]["hr"]
                h = hr[:, t % XB, :]
                nc.vector.tensor_mul(h, st[d]["a"][:, 3 * HE:4 * HE], st[d]["th"])
                st[d]["h"][t] = h
                if t % XB == XB - 1 or t == T - 1:
                    t0 = t - t % XB
                    nc.gpsimd.dma_start(
                        out_d.ap()[d, t0:t + 1].rearrange("t b e -> b t e"),
                        hr[:, 0:t - t0 + 1, :])

            def emit_transpose(d, t):
                if t == T - 1:
                    return
                h = st[d]["h"].pop(t)
                with tc.high_priority(offset=HIPRI):
                    tp = tps.tile([128, B], F32, tag="tp")
                    nc.tensor.transpose(tp, h, ident[0:B, 0:B])
                    hTs = spool.tile([128, B], BF16, tag=f"hTs{d}", name=f"hTs{d}")
                    nc.vector.tensor_copy(hTs, tp)
                    ag_in = dram.tile([128, B], BF16, tag=f"ag_in{d}",
                                      name=f"ag_in{d}")
                    eng = nc.sync if d == 0 else nc.scalar
                    eng.dma_start(ag_in, hTs)
                st[d]["ag_in"] = ag_in

            def emit_ag(d, t):
                if t == T - 1:
                    return
                ag_in = st[d]["ag_in"]
                ag_out = dram.tile([KC_H * 128, B], BF16, tag=f"ag_out{d}",
                                   name=f"ag_out{d}")
                if mode == "noag":
                    # timing stand-in for the AllGather (2 local DMAs;
                    # rows 256+ stay stale — noag is timing-only)
                    nc.sync.dma_start(ag_out[0:128, :], ag_in)
                    nc.sync.dma_start(ag_out[128:256, :], ag_in)
                else:
                    nc.gpsimd.collective_compute(
                        "AllGather", mybir.AluOpType.bypass,
                        replica_groups=[list(range(NC))],
                        ins=[ag_in.opt()], outs=[ag_out.opt()],
                    )
                st[d]["ag_out"] = ag_out

            def emit_hload(d, t):
                if t == T - 1:
                    return
                hT = hpool.tile([128, KC_H, B], BF16, tag=f"hT{d}", name=f"hT{d}")
                with tc.high_priority(offset=HIPRI):
                    eng = nc.sync if d == 0 else nc.scalar
                    eng.dma_start(
                        hT, st[d]["ag_out"].rearrange("(n p) m -> p n m", p=128))
                st[d]["hT"] = hT

            emit_xpart(0, 0)
            emit_xpart(1, 0)
            emit_xpart(0, 1)
            emit_xpart(1, 1)
            for t in range(T):
                emit_hmm(0, t)
                emit_xpart(0, t + 2)
                emit_gates(0, t)
                emit_cup(0, t)
                emit_tanhc(0, t)
                emit_hmm(1, t)
                emit_hmul(0, t)
                emit_transpose(0, t)
                emit_ag(0, t)
                emit_hload(0, t)
                emit_gates(1, t)
                emit_cup(1, t)
                emit_tanhc(1, t)
                emit_hmul(1, t)
                emit_xpart(1, t + 2)
                emit_transpose(1, t)
                emit_ag(1, t)
                emit_hload(1, t)

    nc.compile()
    return nc


def make_in_maps(x, Wk_f, Wr_f, b_f, Wk_b, Wr_b, b_b):
    """Host-side prep: transpose x, slice/reorder weight columns per core."""
    T = x.shape[1]
    xp = np.empty((2, T, D, B), np.float32)
    xp[0] = np.transpose(x, (1, 2, 0))  # [T, D, B]
    xp[1] = xp[0][::-1]
    in_maps = []
    for c in range(NC):
        W = np.zeros((2, W_ROWS, NCOL), np.float32)
        sl = c * HE + np.arange(HE)
        # column order [g, i, f, o]  (keras gate order in W is i,f,g,o)
        cols = np.concatenate([2 * H + sl, 0 * H + sl, 1 * H + sl, 3 * H + sl])
        for d, (Wk, Wr, b) in enumerate([(Wk_f, Wr_f, b_f), (Wk_b, Wr_b, b_b)]):
            W[d, 0:D] = np.asarray(Wk)[:, cols]
            W[d, D] = np.asarray(b)[cols]
            W[d, (KC_X + 1) * 128:] = np.asarray(Wr)[:, cols]
        import ml_dtypes
        wh = W[:, (KC_X + 1) * 128:].astype(ml_dtypes.bfloat16)
        in_maps.append({"xp": xp, "W": W, "Wh": wh,
                        "ones": np.ones((1, B), np.float32)})
    return in_maps


def combine(results, T):
    """Gather per-core [2, T, B, HE] outputs into [B, T, H]."""
    out = np.zeros((B, T, H), np.float32)
    for c in range(NC):
        hs = results[c]["hs"]  # [2, T, B, HE]
        f = np.transpose(hs[0], (1, 0, 2))          # [B, T, HE]
        bwd = np.transpose(hs[1], (1, 0, 2))[:, ::-1]
        out[:, :, c * HE:(c + 1) * HE] = 0.5 * (f + bwd)
    return out


_NC_CACHE = {}


def run(x, Wk_f, Wr_f, b_f, Wk_b, Wr_b, b_b, trace=False, **spmd_kwargs):
    T = x.shape[1]
    has_bias = bool(np.any(b_f) or np.any(b_b))
    key = (T, has_bias)
    if key not in _NC_CACHE:
        _NC_CACHE[key] = build(T, has_bias=has_bias)
    nc = _NC_CACHE[key]
    in_maps = make_in_maps(x, Wk_f, Wr_f, b_f, Wk_b, Wr_b, b_b)
    res = run_bass_kernel_spmd(nc, in_maps, core_ids=list(range(NC)),
                               trace=trace, **spmd_kwargs)
    return combine(res.results, T), res


def kernel(x, Wk_f, Wr_f, b_f, Wk_b, Wr_b, b_b):
    out, _ = run(np.asarray(x), np.asarray(Wk_f), np.asarray(Wr_f), np.asarray(b_f),
                 np.asarray(Wk_b), np.asarray(Wr_b), np.asarray(b_b))
    return out



# revision 4
# speedup vs baseline: 1.6459x; 1.0266x over previous
"""Bidirectional LSTM layer on 8 Trainium2 NeuronCores.

Sharding: each of the 8 cores owns ONE 128-unit eighth of the hidden dim for
BOTH directions (fwd and bwd).  The two directions' recurrences are
independent, so their sub-steps are interleaved: while direction f's serial
tail (gates -> c -> h -> transpose -> AllGather -> reload) is in flight, the
TensorEngine runs direction b's matmuls, and vice versa.  This keeps the PE
continuously busy (warm clock) and hides most of the per-step latency chain.

Per core, per sub-step (direction d, time t):
    z[64, 512] = x_t @ Wk_cols (+ b_cols) + h_{t-1} @ Wr_cols      (PSUM, 1 bank)
with column order [g | i | f | o] (each 128 wide).  h eighth is transposed
on the PE and AllGathered across all 8 cores (one group; rank r contributes
H rows [128r, 128r+128)), yielding the full hT [1024, 64] for the next step.
x-part matmuls are emitted 2 steps ahead (zps bufs=3) as PE filler work.
Matmuls run as float32r (full PE rate); the h recurrence matmuls and the
per-step h exchange (transpose -> AllGather -> reload) run in bfloat16,
halving the collective and reload bytes (~1e-3 rel err, well inside the
2e-2 gate).
"""

import sys

sys.path.insert(0, "/opt/trn_rl_repo")

import numpy as np

import concourse.bass as bass
import concourse.tile as tile
from concourse import bacc, mybir
from concourse.bass_utils import run_bass_kernel_spmd
from concourse.masks import make_identity

F32 = mybir.dt.float32
F32R = mybir.dt.float32r
BF16 = mybir.dt.bfloat16
ACT = mybir.ActivationFunctionType

B = 64       # batch
D = 512      # input dim
H = 1024     # hidden dim
HE = H // 8  # per-core hidden eighth
NCOL = 4 * HE  # 512 z-columns per direction per core
NC = 8

# K-chunk layout of the combined weight matrix W[d] [13*128, 512]:
#   chunks 0..3   : Wk rows (D = 512)
#   chunk  4      : row 0 = bias, rest zero
#   chunks 5..12  : Wr rows (H = 1024)
KC_X = 4
KC_H = 8
W_ROWS = (KC_X + 1 + KC_H) * 128


def build(T: int, mode: str = "full", has_bias: bool = False):
    nc = bacc.Bacc("TRN2", target_bir_lowering=False, debug=False, num_devices=NC)

    xp_d = nc.dram_tensor("xp", [2, T, D, B], F32R, kind="ExternalInput")
    w_d = nc.dram_tensor("W", [2, W_ROWS, NCOL], F32R, kind="ExternalInput")
    wh_d = nc.dram_tensor("Wh", [2, KC_H * 128, NCOL], BF16, kind="ExternalInput")
    ones_d = nc.dram_tensor("ones", [1, B], F32R, kind="ExternalInput")
    out_d = nc.dram_tensor("hs", [2, T, B, HE], F32, kind="ExternalOutput")

    with tile.TileContext(nc) as tc:
        with (
            tc.tile_pool(name="singles", bufs=1) as singles,
            tc.tile_pool(name="xpool", bufs=4) as xpool,
            tc.tile_pool(name="hpool", bufs=3) as hpool,
            tc.tile_pool(name="apool", bufs=3) as apool,
            tc.tile_pool(name="cpool", bufs=2) as cpool,
            tc.tile_pool(name="spool", bufs=4) as spool,
            tc.tile_pool(name="zps", bufs=3, space="PSUM") as zps,
            tc.tile_pool(name="tps", bufs=2, space="PSUM") as tps,
            tc.tile_pool(name="dram", bufs=3, space="DRAM") as dram,
        ):
            # --- persistent tiles ------------------------------------------
            w_sb = singles.tile([128, 2, KC_X + 1 + KC_H, NCOL], F32R)
            nc.sync.dma_start(w_sb, w_d.ap().rearrange("d (n p) m -> p d n m", p=128))

            wh_sb = singles.tile([128, 2, KC_H, NCOL], BF16)
            nc.sync.dma_start(wh_sb,
                              wh_d.ap().rearrange("d (n p) m -> p d n m", p=128))

            ident = singles.tile([128, 128], F32)
            make_identity(nc, ident)

            ones = singles.tile([1, B], F32R)
            nc.sync.dma_start(ones, ones_d.ap())

            st = []  # per-direction state
            for d in range(2):
                c0 = cpool.tile([B, HE], F32, tag=f"c{d}")
                nc.vector.memset(c0, 0.0)
                st.append({"c": c0, "hT": None, "z": {}, "h": {}})

            XB = 4  # steps per batched x load / h store
            HIPRI = 100  # pull exchange-path instructions ahead in the scheduler

            def emit_xpart(d, t):
                """x_t @ Wk (+ bias) into a fresh z psum tile; no h deps."""
                if t >= T:
                    return
                if t % XB == 0:  # batched load of XB steps of x (transposed)
                    nsteps = min(XB, T - t)
                    xT = xpool.tile([128, XB, KC_X, B], F32R, tag=f"xT{d}")
                    nc.gpsimd.dma_start(
                        xT[:, 0:nsteps],
                        xp_d.ap()[d, t:t + nsteps].rearrange(
                            "t (n p) m -> p t n m", p=128))
                    st[d]["xT"] = xT
                xT = st[d]["xT"]
                z = zps.tile([B, NCOL], F32, tag=f"z{d}")
                st[d]["z"][t] = z
                last_x = (t == 0) and not has_bias
                for k in range(KC_X):
                    nc.tensor.matmul(z, xT[:, t % XB, k, :], w_sb[:, d, k, :],
                                     start=(k == 0),
                                     stop=(last_x and k == KC_X - 1))
                if has_bias:
                    nc.tensor.matmul(z, ones[0:1, :], w_sb[0:1, d, KC_X, :],
                                     start=False, stop=(t == 0))

            def emit_hmm(d, t):
                z = st[d]["z"][t]
                if t > 0:
                    hT = st[d]["hT"]
                    for k in range(KC_H):
                        nc.tensor.matmul(z, hT[:, k, :],
                                         wh_sb[:, d, k, :],
                                         start=False, stop=(k == KC_H - 1))

            def emit_gates(d, t):
                """a = [tanh(g), sig(i), sig(f,o)] + ig = i*g."""
                z = st[d]["z"].pop(t)
                a = apool.tile([B, NCOL], F32, tag=f"a{d}", name=f"a{d}")
                nc.scalar.activation(a[:, 0:HE], z[:, 0:HE], ACT.Tanh)
                nc.scalar.activation(a[:, HE:4 * HE], z[:, HE:4 * HE],
                                     ACT.Sigmoid)
                ig = spool.tile([B, HE], F32, tag=f"ig{d}", name=f"ig{d}")
                nc.vector.tensor_mul(ig, a[:, HE:2 * HE], a[:, 0:HE])
                st[d]["a"], st[d]["ig"] = a, ig

            def emit_cup(d, t):
                """c_new = f*c + ig."""
                a = st[d]["a"]
                fc = spool.tile([B, HE], F32, tag=f"fc{d}", name=f"fc{d}")
                nc.vector.tensor_mul(fc, a[:, 2 * HE:3 * HE], st[d]["c"])
                c_new = cpool.tile([B, HE], F32, tag=f"c{d}", name=f"c{d}")
                nc.vector.tensor_add(c_new, fc, st[d]["ig"])
                st[d]["c"] = c_new

            def emit_tanhc(d, t):
                th = spool.tile([B, HE], F32, tag=f"th{d}", name=f"th{d}")
                nc.scalar.activation(th, st[d]["c"], ACT.Tanh)
                st[d]["th"] = th

            def emit_hmul(d, t):
                if t % XB == 0:  # h ring buffer, stored XB steps at a time
                    st[d]["hr"] = spool.tile([B, XB, HE], F32, tag=f"hr{d}",
                                             name=f"hr{d}")
                hr = st[d]["hr"]
                h = hr[:, t % XB, :]
                nc.vector.tensor_mul(h, st[d]["a"][:, 3 * HE:4 * HE], st[d]["th"])
                st[d]["h"][t] = h
                if t % XB == XB - 1 or t == T - 1:
                    t0 = t - t % XB
                    nc.gpsimd.dma_start(
                        out_d.ap()[d, t0:t + 1].rearrange("t b e -> b t e"),
                        hr[:, 0:t - t0 + 1, :])

            def emit_transpose(d, t):
                if t == T - 1:
                    return
                h = st[d]["h"].pop(t)
                with tc.high_priority(offset=HIPRI):
                    tp = tps.tile([128, B], F32, tag="tp")
                    nc.tensor.transpose(tp, h, ident[0:B, 0:B])
                    hTs = spool.tile([128, B], BF16, tag=f"hTs{d}", name=f"hTs{d}")
                    nc.vector.tensor_copy(hTs, tp)
                    ag_in = dram.tile([128, B], BF16, tag=f"ag_in{d}",
                                      name=f"ag_in{d}")
                    eng = nc.sync if d == 0 else nc.scalar
                    eng.dma_start(ag_in, hTs)
                st[d]["ag_in"] = ag_in

            def emit_ag(d, t):
                if t == T - 1:
                    return
                ag_in = st[d]["ag_in"]
                ag_out = dram.tile([KC_H * 128, B], BF16, tag=f"ag_out{d}",
                                   name=f"ag_out{d}")
                if mode == "noag":
                    # timing stand-in for the AllGather (2 local DMAs;
                    # rows 256+ stay stale — noag is timing-only)
                    nc.sync.dma_start(ag_out[0:128, :], ag_in)
                    nc.sync.dma_start(ag_out[128:256, :], ag_in)
                else:
                    nc.gpsimd.collective_compute(
                        "AllGather", mybir.AluOpType.bypass,
                        replica_groups=[list(range(NC))],
                        ins=[ag_in.opt()], outs=[ag_out.opt()],
                    )
                st[d]["ag_out"] = ag_out

            def emit_hload(d, t):
                if t == T - 1:
                    return
                hT = hpool.tile([128, KC_H, B], BF16, tag=f"hT{d}", name=f"hT{d}")
                with tc.high_priority(offset=HIPRI):
                    eng = nc.sync if d == 0 else nc.scalar
                    eng.dma_start(
                        hT, st[d]["ag_out"].rearrange("(n p) m -> p n m", p=128))
                st[d]["hT"] = hT

            emit_xpart(0, 0)
            emit_xpart(1, 0)
            emit_xpart(0, 1)
            emit_xpart(1, 1)
            for t in range(T):
                emit_hmm(0, t)
                emit_xpart(0, t + 2)
                emit_gates(0, t)
                emit_cup(0, t)
                emit_tanhc(0, t)
                emit_hmm(1, t)
                emit_hmul(0, t)
                emit_transpose(0, t)
                emit_ag(0, t)
                emit_hload(0, t)
                emit_gates(1, t)
                emit_cup(1, t)
                emit_tanhc(1, t)
                emit_hmul(1, t)
                emit_xpart(1, t + 2)
                emit_transpose(1, t)
                emit_ag(1, t)
                emit_hload(1, t)

    nc.compile()
    return nc


def make_in_maps(x, Wk_f, Wr_f, b_f, Wk_b, Wr_b, b_b):
    """Host-side prep: transpose x, slice/reorder weight columns per core."""
    T = x.shape[1]
    xp = np.empty((2, T, D, B), np.float32)
    xp[0] = np.transpose(x, (1, 2, 0))  # [T, D, B]
    xp[1] = xp[0][::-1]
    in_maps = []
    for c in range(NC):
        W = np.zeros((2, W_ROWS, NCOL), np.float32)
        sl = c * HE + np.arange(HE)
        # column order [g, i, f, o]  (keras gate order in W is i,f,g,o)
        cols = np.concatenate([2 * H + sl, 0 * H + sl, 1 * H + sl, 3 * H + sl])
        for d, (Wk, Wr, b) in enumerate([(Wk_f, Wr_f, b_f), (Wk_b, Wr_b, b_b)]):
            W[d, 0:D] = np.asarray(Wk)[:, cols]
            W[d, D] = np.asarray(b)[cols]
            W[d, (KC_X + 1) * 128:] = np.asarray(Wr)[:, cols]
        import ml_dtypes
        wh = W[:, (KC_X + 1) * 128:].astype(ml_dtypes.bfloat16)
        in_maps.append({"xp": xp, "W": W, "Wh": wh,
                        "ones": np.ones((1, B), np.float32)})
    return in_maps


def combine(results, T):
    """Gather per-core [2, T, B, HE] outputs into [B, T, H]."""
    out = np.zeros((B, T, H), np.float32)
    for c in range(NC):
        hs = results[c]["hs"]  # [2, T, B, HE]
        f = np.transpose(hs[0], (1, 0, 2))          # [B, T, HE]
        bwd = np.transpose(hs[1], (1, 0, 2))[:, ::-1]
        out[:, :, c * HE:(c + 1) * HE] = 0.5 * (f + bwd)
    return out


_NC_CACHE = {}


def run(x, Wk_f, Wr_f, b_f, Wk_b, Wr_b, b_b, trace=False, **spmd_kwargs):
    T = x.shape[1]
    has_bias = bool(np.any(b_f) or np.any(b_b))
    key = (T, has_bias)
    if key not in _NC_CACHE:
        _NC_CACHE[key] = build(T, has_bias=has_bias)
    nc = _NC_CACHE[key]
    in_maps = make_in_maps(x, Wk_f, Wr_f, b_f, Wk_b, Wr_b, b_b)
    res = run_bass_kernel_spmd(nc, in_maps, core_ids=list(range(NC)),
                               trace=trace, **spmd_kwargs)
    return combine(res.results, T), res


def kernel(x, Wk_f, Wr_f, b_f, Wk_b, Wr_b, b_b):
    out, _ = run(np.asarray(x), np.asarray(Wk_f), np.asarray(Wr_f), np.asarray(b_f),
                 np.asarray(Wk_b), np.asarray(Wr_b), np.asarray(b_b))
    return out

